# revision 39
# baseline (speedup 1.0000x reference)
"""AtomicBasis GNN message-passing kernel for 8 TRN2 NeuronCores (v3).

A[k,x,y,z] = sum_a  c*sin(k*pi*d_a/5)/d_a * (h@W.T)[a,k] * nx*ny*nz
with n = rel_pos/d.  Rewritten as  A = sum_a w[a,k] * m[a,s] where
  w[a,k] = sin(2*pi*frac(k*d_a/10)) * (h@W.T)[a,k]
  m[a,s] = monomial_s(rp) * c/d^4     (s = 10 distinct symmetric monomials)
Shard a across 8 cores (data parallel); sum the (128,20) partials on host.

v3: GPSIMD is banned from the main loop (its software tensor ops saturate
SBUF bandwidth and throttle concurrent DVE ops 3-12x, measured).  The sin
arguments are produced on the TENSOR engine instead:

  PE-argu: per 8-q block, one bf16 matmul
      t[c, 64q+k] = sum_j lhsT[j,c] * KD[j, 64q+k]
  with lhsT = [theta_c rows(8) | theta_r rows(8) | ones row | pad(15)]
  (a transposed, interleaved view of theta built once via PE transposes;
  windows sit at 32-aligned base partitions as the PE requires) and KD
  the constant rhs holding (k+1)-diagonals for the coarse/residual theta
  rows plus a 192.0 magic row, replicated at partitions 0/32/64/96.  theta = theta_c + theta_r is the
  bf16 split of d/10 (phase error ~1e-3 rad).  The f32 accumulator
  t = k*theta + 192 lands in [192, 256) whose ulp is 2^-16, so the LOW 16
  BITS of the f32 word are exactly frac(k*theta)*2^16: the Sin ACT reads
  them as int16 via a stride-2 halfword view of PSUM (free mod 2pi) and
  writes k-fast sinb tiles.  Requires d < 10 (holds, d <= ~7).

  fold w = sinb*hp per 16-q unit (1024 elems): 'x' direct DVE (PSUM read,
  1x) or 'z' ACT-copy->SBUF bf16 + DVE 2x, pattern-balanced.
  MMW: PE matmul lhsT=hT-pair (pair-stacked host bf16 layout),
  rhs=blkdiag W.  MMA: PE lhsT=w-pair, rhs=m10-pair accumulated in one
  PSUM bank over all 992 pairs.  c/d^4 via ACT Ln+Exp.
  PSUM: one rotating pool of [128,1024] tiles (AG phase tiles + hp tiles,
  3 bufs = 6 banks) + 1 accumulator bank.
"""

import os
import sys
import numpy as np

for _p in ("/opt/trn_rl_repo", "/root/problem/trn_rl_repo"):
    if os.path.isdir(_p) and _p not in sys.path:
        sys.path.insert(0, _p)

import ml_dtypes

N_GLOBAL = 1_000_000
K = 64
P = 128
Q = 992                      # q's (a-columns) per partition per core
NLOC = P * Q                 # 126976 per core
NCORES = 8
NTOT = NCORES * NLOC         # 1015808 >= 1e6 (padded)
R_CUT = 5.0
C_RBF = float(np.sqrt(2.0 / R_CUT))

NUNIT = Q // 16              # 62 main-loop units of 16 q (8 pairs)
NBLK = Q // 8                # 124 8-q phase blocks
NW = NBLK // 2               # 62 transpose windows (2 blocks at bases 0/64)
FIX16 = 65536.0
SC16 = float((2.0 ** 16 / (2.0 * R_CUT)) ** 2)   # thf = sqrt(d^2*SC16) = d/10*2^16
MAGIC = 192.0

# fold path per unit: 'x' direct DVE (1x), 'z' ACT-copy + DVE 2x
FOLD_PATTERN = "x"

# s-index -> monomial: s = 3*alpha+beta is rp[alpha]^2*rp[beta]*q2 (s 0..8),
# s=9 is x*y*z*q2. Host expands 10 -> 27 via sorted-multiset lookup.
_MONO = {}
for _a in range(3):
    for _b in range(3):
        _MONO.setdefault(tuple(sorted([_a, _a, _b])), 3 * _a + _b)
_MONO[(0, 1, 2)] = 9

_CACHE = {}


def _build_nc():
    import concourse.bass as bass
    import concourse.bacc as bacc
    import concourse.tile as tile
    import concourse.mybir as mybir

    f32 = mybir.dt.float32
    bf16 = mybir.dt.bfloat16
    i16 = mybir.dt.int16

    nc = bacc.Bacc(
        "TRN2",
        target_bir_lowering=False,
        debug=False,
        enable_asserts=True,
        num_devices=NCORES,
    )

    HT_COLS = (Q // 2) * P
    htp_ext = nc.dram_tensor("htp", [P, HT_COLS], bf16, kind="ExternalInput").ap()
    rp_ext = nc.dram_tensor("rp", [3, NLOC], f32, kind="ExternalInput").ap()
    blkw_ext = nc.dram_tensor("blkw", [P, P], bf16, kind="ExternalInput").ap()
    kd_ext = nc.dram_tensor("kd", [P, 512], bf16, kind="ExternalInput").ap()
    idn_ext = nc.dram_tensor("idn", [P, P], f32, kind="ExternalInput").ap()
    out_ext = nc.dram_tensor("out", [P, 20], f32, kind="ExternalOutput").ap()

    SIN = mybir.ActivationFunctionType.Sin
    SQRT = mybir.ActivationFunctionType.Sqrt
    LN = mybir.ActivationFunctionType.Ln
    EXP = mybir.ActivationFunctionType.Exp
    COPY = mybir.ActivationFunctionType.Copy

    with tile.TileContext(nc) as tc:
        from contextlib import ExitStack

        with ExitStack() as ctx:
            # ---- persistent pool ----
            const = ctx.enter_context(tc.tile_pool(name="const", bufs=1))
            blkw = const.tile([P, P], bf16)
            kd = const.tile([P, 512], bf16)
            zcol = const.tile([P, 1], f32)
            lncol = const.tile([P, 1], f32)
            m10 = const.tile([P, 10 * Q], bf16)
            m10v = m10[:].rearrange("p (s q) -> p s q", s=10)
            tmx = const.tile([P, NW * P], bf16)   # transposed theta windows

            nc.sync.dma_start(blkw[:], blkw_ext)
            nc.sync.dma_start(kd[:], kd_ext)
            nc.vector.memset(zcol[:], 0.0)
            nc.vector.memset(lncol[:], float(np.log(C_RBF)))

            # ---- main pools ----
            hpool = ctx.enter_context(tc.tile_pool(name="hch", bufs=10))
            sinp = ctx.enter_context(tc.tile_pool(name="sin", bufs=6))
            wpool = ctx.enter_context(tc.tile_pool(name="wf", bufs=6))
            hsb = ctx.enter_context(tc.tile_pool(name="hsb", bufs=3))
            agp = ctx.enter_context(
                tc.tile_pool(name="agp", bufs=3, space=bass.MemorySpace.PSUM)
            )
            php = ctx.enter_context(
                tc.tile_pool(name="php", bufs=2, space=bass.MemorySpace.PSUM)
            )
            psA = ctx.enter_context(
                tc.tile_pool(name="psA", bufs=1, space=bass.MemorySpace.PSUM)
            )
            thp = ctx.enter_context(tc.tile_pool(name="thp", bufs=1))
            thi = thp.tile([P, 64 * NBLK], f32)
            idn = thp.tile([P, P], f32)

            A_ps = psA.tile([P, 20], f32)

            hT_tiles = {}
            state = {}
            tmx_done = set()
            tr_src = {}

            def ensure_granule(g):
                if g not in hT_tiles:
                    t = hpool.tile([P, 2048], bf16, tag="hT")
                    nc.sync.dma_start(t[:], htp_ext[:, g * 2048 : (g + 1) * 2048])
                    hT_tiles[g] = t
                return hT_tiles[g]

            def produce_mmw(u):
                gran = ensure_granule(u // 2)
                base = (u % 2) * 8
                hp = php.tile([P, 1024], f32, tag="hp")
                for t in range(8):
                    nc.tensor.matmul(
                        hp[:, 128 * t : 128 * (t + 1)],
                        gran[:, 128 * (base + t) : 128 * (base + t + 1)],
                        blkw[:],
                        start=True,
                        stop=True,
                        skip_group_check=True,
                    )
                state[u] = hp

            def ensure_window(w):
                # lazy PE transpose of theta window w into tmx
                if w in tmx_done:
                    return
                tmx_done.add(w)
                thi = tr_src["thi"]
                idn = tr_src["idn"]
                tt = agp.tile([P, 512], f32, tag="ag")
                nc.tensor.transpose(
                    tt[:, 0:P], thi[:, w * P : (w + 1) * P], idn[:]
                )
                nc.scalar.activation(
                    tmx[:, w * P : (w + 1) * P], tt[:, 0:P], COPY,
                    bias=0.0, scale=1.0,
                )

            def produce_phase(u):
                ensure_window(u)
                if u + 1 < NW:
                    ensure_window(u + 1)
                sb = sinp.tile([P, 1024], bf16, tag="sb")
                for s in range(2):
                    blk = 2 * u + s
                    w, win = divmod(blk, 2)
                    ag = agp.tile([P, 512], f32, tag="ag")
                    nc.tensor.matmul(
                        ag[:],
                        tmx[64 * win : 64 * win + 17, w * P : (w + 1) * P],
                        kd[64 * win : 64 * win + 17, :],
                        start=True,
                        stop=True,
                        skip_group_check=True,
                    )
                    # sin from the low halfwords of the f32 phase words
                    nc.scalar.activation(
                        sb[:, 512 * s : 512 * (s + 1)],
                        ag[:]
                        .bitcast(i16)
                        .rearrange("p (n t) -> p n t", t=2)[:, :, 0],
                        SIN,
                        bias=zcol[:],
                        scale=float(2.0 * np.pi / FIX16),
                    )
                state[u] = (state[u], sb)

            def fold_mma_unit(u):
                hp, sb = state.pop(u)
                path = FOLD_PATTERN[u % len(FOLD_PATTERN)]
                w = wpool.tile([P, 1024], bf16, tag="w")
                if path == "x":
                    nc.vector.tensor_mul(w[:], sb[:], hp[:])
                else:
                    hs = hsb.tile([P, 1024], bf16, tag="hs")
                    nc.scalar.activation(hs[:], hp[:], COPY, bias=0.0, scale=1.0)
                    nc.vector.tensor_mul(w[:], sb[:], hs[:])
                for t in range(8):
                    gp = 8 * u + t
                    nc.tensor.matmul(
                        A_ps[:],
                        w[:, 128 * t : 128 * (t + 1)],
                        m10v[:, :, 2 * gp : 2 * gp + 2],
                        start=(gp == 0),
                        stop=(gp == 8 * NUNIT - 1),
                        skip_group_check=True,
                    )

            # ---- prologue (scoped transients) ----
            with tc.tile_pool(name="prol", bufs=1) as prol:
                # prefetch the first hT granules on the sync DMA queue before
                # anything big; rp/idn go on the scalar queue (parallel)
                for g in range(4):
                    ensure_granule(g)
                nc.scalar.dma_start(idn[:], idn_ext)
                rp_all = prol.tile([P, 3 * Q], f32)
                nc.scalar.dma_start(
                    rp_all[:].rearrange("p (x q) -> p x q", x=3),
                    rp_ext.rearrange("x (p q) -> p x q", p=P),
                )
                rx = rp_all[:, 0 * Q : 1 * Q]
                ry = rp_all[:, 1 * Q : 2 * Q]
                rz = rp_all[:, 2 * Q : 3 * Q]
                rp3 = rp_all[:].rearrange("p (x q) -> p x q", x=3)

                t_a = prol.tile([P, Q], f32)
                t_b = prol.tile([P, Q], f32)
                d2 = prol.tile([P, Q], f32)
                nc.vector.tensor_mul(t_a[:], rx, rx)
                nc.vector.tensor_mul(t_b[:], ry, ry)
                nc.vector.tensor_add(t_a[:], t_a[:], t_b[:])
                nc.vector.tensor_mul(t_b[:], rz, rz)
                nc.vector.tensor_add(d2[:], t_a[:], t_b[:])

                # theta_turns = d/10 (f32), split into bf16 coarse+residual
                thf = prol.tile([P, Q], f32)
                nc.scalar.activation(
                    thf[:], d2[:], SQRT, bias=zcol[:], scale=float(SC16 / FIX16**2)
                )
                thc = prol.tile([P, Q], bf16)
                nc.vector.tensor_copy(thc[:], thf[:])
                thr = prol.tile([P, Q], bf16)
                nc.vector.tensor_sub(thr[:], thf[:], thc[:])

                # interleaved [thc(8) | thr(8) | 1 | pad] per 8-q block, then
                # PE transposes into tmx.  Pad rows 17-63 of each window are
                # never read by the phase-matmul lhsT AP: left uninitialised.
                thiv = thi[:].rearrange("p (b j) -> p b j", j=64)
                nc.vector.tensor_copy(
                    thiv[:, :, 0:8], thc[:].rearrange("p (b j) -> p b j", j=8)
                )
                nc.vector.tensor_copy(
                    thiv[:, :, 8:16], thr[:].rearrange("p (b j) -> p b j", j=8)
                )
                nc.vector.memset(thiv[:, :, 16:17], 1.0)

                # warm the PE / overlap: first two MMW units only need h+W
                produce_mmw(0)
                produce_mmw(1)
                tr_src["thi"] = thi
                tr_src["idn"] = idn

                # q2 = c/d^4 = exp(-2*ln(d2) + ln(c)); reuse t_a/t_b
                lnq = t_a
                nc.scalar.activation(lnq[:], d2[:], LN, bias=zcol[:], scale=1.0)
                q2 = t_b
                nc.scalar.activation(q2[:], lnq[:], EXP, bias=lncol[:], scale=-2.0)

                rp_s = prol.tile([P, 3 * Q], f32)        # rp * (c/d^4)
                rps3 = rp_s[:].rearrange("p (x q) -> p x q", x=3)
                nc.vector.tensor_mul(
                    rps3, rp3, q2[:].unsqueeze(1).broadcast_to((P, 3, Q))
                )
                sq_s = prol.tile([P, 3 * Q], f32)        # rp^2 * (c/d^4)
                sqs3 = sq_s[:].rearrange("p (x q) -> p x q", x=3)
                nc.vector.tensor_mul(sqs3, rp3, rps3)
                xyq = d2                                 # x*y*(c/d^4), reuse d2
                nc.vector.tensor_mul(xyq[:], rx, rp_s[:, 1 * Q : 2 * Q])

                # m10: 10 plain (P,Q) muls, all on DVE (GPSIMD tensor ops
                # saturate SBUF bandwidth and throttle concurrent DVE 3-12x)
                for s in range(9):
                    al, be = divmod(s, 3)
                    nc.vector.tensor_mul(
                        m10[:, s * Q : (s + 1) * Q],
                        sq_s[:, al * Q : (al + 1) * Q],
                        rp_all[:, be * Q : (be + 1) * Q],
                    )
                nc.vector.tensor_mul(m10[:, 9 * Q : 10 * Q], xyq[:], rz)

            # ---- main loop: 2-deep software pipeline ----
            produce_phase(0)
            produce_phase(1)
            for u in range(2, NUNIT):
                produce_mmw(u)
                produce_phase(u)
                fold_mma_unit(u - 2)
            fold_mma_unit(NUNIT - 2)
            fold_mma_unit(NUNIT - 1)

            # ---- epilogue ----
            A_sb = const.tile([P, 20], f32)
            nc.vector.tensor_copy(A_sb[:], A_ps[:])
            nc.gpsimd.dma_start(out_ext, A_sb[:])

    nc.compile()
    return nc


def _get_nc():
    if "nc" not in _CACHE:
        _CACHE["nc"] = _build_nc()
    return _CACHE["nc"]


def kernel(h, rel_poss, W):
    from concourse.bass_utils import run_bass_kernel_spmd

    nc = _get_nc()

    h_pad = np.zeros((NTOT, K), dtype=np.float32)
    h_pad[:N_GLOBAL] = h
    rp_pad = np.ones((3, NTOT), dtype=np.float32)
    rp_pad[:, :N_GLOBAL] = rel_poss

    # Pre-transpose h to pair-stacked bf16 layout:
    # htp[i, 64*o + j, pi*128 + c] = h[i*NLOC + c*Q + 2*pi + o, j]
    Hc = h_pad.reshape(NCORES, P, Q, K).astype(ml_dtypes.bfloat16)
    ht = Hc.transpose(0, 3, 2, 1)                     # (i, j, q, c)
    htp = np.ascontiguousarray(
        ht.reshape(NCORES, K, Q // 2, 2, P).transpose(0, 3, 1, 2, 4)
    ).reshape(NCORES, P, (Q // 2) * P)

    wt = np.ascontiguousarray(W.T.astype(np.float32))   # wt[j,k] = W[k,j]
    blkw = np.zeros((P, P), dtype=np.float32)
    blkw[0:K, 0:K] = wt
    blkw[K:P, K:P] = wt
    blkw = blkw.astype(ml_dtypes.bfloat16)

    # KD: [17, 512] rhs for the PE phase matmul.
    # rows 0-7: (k+1)-diagonal for theta_c; rows 8-15: same for theta_r;
    # row 16: the 192.0 magic constant.
    kdm = np.zeros((P, 512), dtype=np.float32)
    kvals = np.arange(1, K + 1, dtype=np.float32)
    for base in (0, 64):
        for q in range(8):
            kdm[base + q, 64 * q : 64 * q + 64] = kvals
            kdm[base + 8 + q, 64 * q : 64 * q + 64] = kvals
        kdm[base + 16, :] = MAGIC
    kdm = kdm.astype(ml_dtypes.bfloat16)

    idn = np.eye(P, dtype=np.float32)

    in_maps = []
    for i in range(NCORES):
        in_maps.append(
            {
                "htp": htp[i],
                "rp": np.ascontiguousarray(rp_pad[:, i * NLOC : (i + 1) * NLOC]),
                "blkw": blkw,
                "kd": kdm,
                "idn": idn,
            }
        )

    res = run_bass_kernel_spmd(
        nc, in_maps, core_ids=list(range(NCORES)), trace=_CACHE.get("trace", False)
    )
    _CACHE["last_results"] = res
    acc = np.sum(
        [np.asarray(res.results[i]["out"], dtype=np.float32) for i in range(NCORES)],
        axis=0,
    )                                               # (128, 20)
    a20 = acc.reshape(P, 10, 2)
    A10 = a20[0:K, :, 0] + a20[K:P, :, 1]           # (64, 10)

    A = np.empty((K, 3, 3, 3), dtype=np.float32)
    for x in range(3):
        for y in range(3):
            for z in range(3):
                A[:, x, y, z] = A10[:, _MONO[tuple(sorted((x, y, z)))]]
    return A


if __name__ == "__main__":
    nc = _get_nc()
    print("build + compile OK")


# revision 40
# speedup vs baseline: 1.3593x; 1.3593x over previous
"""AtomicBasis GNN message-passing kernel for 8 TRN2 NeuronCores (v3).

A[k,x,y,z] = sum_a  c*sin(k*pi*d_a/5)/d_a * (h@W.T)[a,k] * nx*ny*nz
with n = rel_pos/d.  Rewritten as  A = sum_a w[a,k] * m[a,s] where
  w[a,k] = sin(2*pi*frac(k*d_a/10)) * (h@W.T)[a,k]
  m[a,s] = monomial_s(rp) * c/d^4     (s = 10 distinct symmetric monomials)
Shard a across 8 cores (data parallel); sum the (128,20) partials on host.

v3: GPSIMD is banned from the main loop (its software tensor ops saturate
SBUF bandwidth and throttle concurrent DVE ops 3-12x, measured).  The sin
arguments are produced on the TENSOR engine instead:

  PE-argu: per 8-q block, one bf16 matmul
      t[c, 64q+k] = sum_j lhsT[j,c] * KD[j, 64q+k]
  with lhsT = [theta_c rows(8) | theta_r rows(8) | ones row | pad(15)]
  (a transposed, interleaved view of theta built once via PE transposes;
  windows sit at 32-aligned base partitions as the PE requires) and KD
  the constant rhs holding (k+1)-diagonals for the coarse/residual theta
  rows plus a 192.0 magic row, replicated at partitions 0/32/64/96.  theta = theta_c + theta_r is the
  bf16 split of d/10 (phase error ~1e-3 rad).  The f32 accumulator
  t = k*theta + 192 lands in [192, 256) whose ulp is 2^-16, so the LOW 16
  BITS of the f32 word are exactly frac(k*theta)*2^16: the Sin ACT reads
  them as int16 via a stride-2 halfword view of PSUM (free mod 2pi) and
  writes k-fast sinb tiles.  Requires d < 10 (holds, d <= ~7).

  fold w = sinb*hp per 16-q unit (1024 elems): 'x' direct DVE (PSUM read,
  1x) or 'z' ACT-copy->SBUF bf16 + DVE 2x, pattern-balanced.
  MMW: PE matmul lhsT=hT-pair (pair-stacked host bf16 layout),
  rhs=blkdiag W.  MMA: PE lhsT=w-pair, rhs=m10-pair accumulated in one
  PSUM bank over all 992 pairs.  c/d^4 via ACT Ln+Exp.
  PSUM: one rotating pool of [128,1024] tiles (AG phase tiles + hp tiles,
  3 bufs = 6 banks) + 1 accumulator bank.
"""

import os
import sys
import numpy as np

for _p in ("/opt/trn_rl_repo", "/root/problem/trn_rl_repo"):
    if os.path.isdir(_p) and _p not in sys.path:
        sys.path.insert(0, _p)

import ml_dtypes

N_GLOBAL = 1_000_000
K = 64
P = 128
Q = 992                      # q's (a-columns) per partition per core
NLOC = P * Q                 # 126976 per core
NCORES = 8
NTOT = NCORES * NLOC         # 1015808 >= 1e6 (padded)
R_CUT = 5.0
C_RBF = float(np.sqrt(2.0 / R_CUT))

NUNIT = Q // 16              # 62 main-loop units of 16 q (8 pairs)
NBLK = Q // 8                # 124 8-q phase blocks
NW = NBLK // 2               # 62 transpose windows (2 blocks at bases 0/64)
FIX16 = 65536.0
SC16 = float((2.0 ** 16 / (2.0 * R_CUT)) ** 2)   # thf = sqrt(d^2*SC16) = d/10*2^16
MAGIC = 192.0

# fold path per unit: 'x' direct DVE (1x), 'z' ACT-copy + DVE 2x
FOLD_PATTERN = "x"

# s-index -> monomial: s = 3*alpha+beta is rp[alpha]^2*rp[beta]*q2 (s 0..8),
# s=9 is x*y*z*q2. Host expands 10 -> 27 via sorted-multiset lookup.
_MONO = {}
for _a in range(3):
    for _b in range(3):
        _MONO.setdefault(tuple(sorted([_a, _a, _b])), 3 * _a + _b)
_MONO[(0, 1, 2)] = 9

_CACHE = {}


def _build_nc():
    import concourse.bass as bass
    import concourse.bacc as bacc
    import concourse.tile as tile
    import concourse.mybir as mybir

    f32 = mybir.dt.float32
    bf16 = mybir.dt.bfloat16
    i16 = mybir.dt.int16

    nc = bacc.Bacc(
        "TRN2",
        target_bir_lowering=False,
        debug=False,
        enable_asserts=True,
        num_devices=NCORES,
    )

    HT_COLS = (Q // 2) * P
    htp_ext = nc.dram_tensor("htp", [P, HT_COLS], bf16, kind="ExternalInput").ap()
    rp_ext = nc.dram_tensor("rp", [3, NLOC], f32, kind="ExternalInput").ap()
    blkw_ext = nc.dram_tensor("blkw", [P, P], bf16, kind="ExternalInput").ap()
    kd_ext = nc.dram_tensor("kd", [P, 512], bf16, kind="ExternalInput").ap()
    idn_ext = nc.dram_tensor("idn", [P, P], f32, kind="ExternalInput").ap()
    out_ext = nc.dram_tensor("out", [P, 20], f32, kind="ExternalOutput").ap()

    SIN = mybir.ActivationFunctionType.Sin
    SQRT = mybir.ActivationFunctionType.Sqrt
    LN = mybir.ActivationFunctionType.Ln
    EXP = mybir.ActivationFunctionType.Exp
    COPY = mybir.ActivationFunctionType.Copy

    with tile.TileContext(nc) as tc:
        from contextlib import ExitStack

        with ExitStack() as ctx:
            # ---- persistent pool ----
            const = ctx.enter_context(tc.tile_pool(name="const", bufs=1))
            blkw = const.tile([P, P], bf16)
            kd = const.tile([P, 512], bf16)
            zcol = const.tile([P, 1], f32)
            lncol = const.tile([P, 1], f32)
            m10 = const.tile([P, 10 * Q], bf16)
            m10v = m10[:].rearrange("p (s q) -> p s q", s=10)
            tmx = const.tile([P, NW * P], bf16)   # transposed theta windows

            nc.sync.dma_start(blkw[:], blkw_ext)
            nc.sync.dma_start(kd[:], kd_ext)
            nc.vector.memset(zcol[:], 0.0)
            nc.vector.memset(lncol[:], float(np.log(C_RBF)))

            # ---- main pools ----
            hpool = ctx.enter_context(tc.tile_pool(name="hch", bufs=10))
            sinp = ctx.enter_context(tc.tile_pool(name="sin", bufs=6))
            wpool = ctx.enter_context(tc.tile_pool(name="wf", bufs=6))
            hsb = ctx.enter_context(tc.tile_pool(name="hsb", bufs=3))
            agp = ctx.enter_context(
                tc.tile_pool(name="agp", bufs=3, space=bass.MemorySpace.PSUM)
            )
            php = ctx.enter_context(
                tc.tile_pool(name="php", bufs=2, space=bass.MemorySpace.PSUM)
            )
            psA = ctx.enter_context(
                tc.tile_pool(name="psA", bufs=1, space=bass.MemorySpace.PSUM)
            )
            thp = ctx.enter_context(tc.tile_pool(name="thp", bufs=1))
            thi = thp.tile([P, 64 * NBLK], f32)
            idn = thp.tile([P, P], f32)

            A_ps = psA.tile([P, 20], f32)

            hT_tiles = {}
            state = {}
            tmx_done = set()
            tr_src = {}

            def ensure_granule(g):
                if g not in hT_tiles:
                    t = hpool.tile([P, 2048], bf16, tag="hT")
                    nc.sync.dma_start(t[:], htp_ext[:, g * 2048 : (g + 1) * 2048])
                    hT_tiles[g] = t
                return hT_tiles[g]

            def produce_mmw(u):
                gran = ensure_granule(u // 2)
                base = (u % 2) * 8
                hp = php.tile([P, 1024], f32, tag="hp")
                for t in range(8):
                    nc.tensor.matmul(
                        hp[:, 128 * t : 128 * (t + 1)],
                        gran[:, 128 * (base + t) : 128 * (base + t + 1)],
                        blkw[:],
                        start=True,
                        stop=True,
                        skip_group_check=True,
                    )
                state[u] = hp

            def ensure_window(w):
                # lazy PE transpose of theta window w into tmx
                if w in tmx_done:
                    return
                tmx_done.add(w)
                thi = tr_src["thi"]
                idn = tr_src["idn"]
                tt = agp.tile([P, 512], f32, tag="ag")
                nc.tensor.transpose(
                    tt[:, 0:P], thi[:, w * P : (w + 1) * P], idn[:]
                )
                nc.scalar.activation(
                    tmx[:, w * P : (w + 1) * P], tt[:, 0:P], COPY,
                    bias=0.0, scale=1.0,
                )

            def produce_phase(u):
                ensure_window(u)
                if u + 1 < NW:
                    ensure_window(u + 1)
                sb = sinp.tile([P, 1024], bf16, tag="sb")
                for s in range(2):
                    blk = 2 * u + s
                    w, win = divmod(blk, 2)
                    ag = agp.tile([P, 512], f32, tag="ag")
                    nc.tensor.matmul(
                        ag[:],
                        tmx[64 * win : 64 * win + 17, w * P : (w + 1) * P],
                        kd[64 * win : 64 * win + 17, :],
                        start=True,
                        stop=True,
                        skip_group_check=True,
                    )
                    # sin from the low halfwords of the f32 phase words
                    nc.scalar.activation(
                        sb[:, 512 * s : 512 * (s + 1)],
                        ag[:]
                        .bitcast(i16)
                        .rearrange("p (n t) -> p n t", t=2)[:, :, 0],
                        SIN,
                        bias=zcol[:],
                        scale=float(2.0 * np.pi / FIX16),
                    )
                state[u] = (state[u], sb)

            def fold_unit(u):
                hp, sb = state.pop(u)
                path = FOLD_PATTERN[u % len(FOLD_PATTERN)]
                w = wpool.tile([P, 1024], bf16, tag="w")
                if path == "x":
                    nc.vector.tensor_mul(w[:], sb[:], hp[:])
                else:
                    hs = hsb.tile([P, 1024], bf16, tag="hs")
                    nc.scalar.activation(hs[:], hp[:], COPY, bias=0.0, scale=1.0)
                    nc.vector.tensor_mul(w[:], sb[:], hs[:])
                return w

            def mma_unit(u, w):
                for t in range(8):
                    gp = 8 * u + t
                    nc.tensor.matmul(
                        A_ps[:],
                        w[:, 128 * t : 128 * (t + 1)],
                        m10v[:, :, 2 * gp : 2 * gp + 2],
                        start=(gp == 0),
                        stop=(gp == 8 * NUNIT - 1),
                        skip_group_check=True,
                    )

            def fold_mma_unit(u):
                mma_unit(u, fold_unit(u))

            # ---- prologue (scoped transients) ----
            with tc.tile_pool(name="prol", bufs=1) as prol:
                # prefetch the first hT granules on the sync DMA queue before
                # anything big; rp/idn go on the scalar queue (parallel)
                for g in range(4):
                    ensure_granule(g)
                nc.scalar.dma_start(idn[:], idn_ext)
                rp_all = prol.tile([P, 3 * Q], f32)
                nc.scalar.dma_start(
                    rp_all[:].rearrange("p (x q) -> p x q", x=3),
                    rp_ext.rearrange("x (p q) -> p x q", p=P),
                )
                rx = rp_all[:, 0 * Q : 1 * Q]
                ry = rp_all[:, 1 * Q : 2 * Q]
                rz = rp_all[:, 2 * Q : 3 * Q]
                rp3 = rp_all[:].rearrange("p (x q) -> p x q", x=3)

                t_a = prol.tile([P, Q], f32)
                t_b = prol.tile([P, Q], f32)
                d2 = prol.tile([P, Q], f32)
                nc.vector.tensor_mul(t_a[:], rx, rx)
                nc.vector.tensor_mul(t_b[:], ry, ry)
                nc.vector.tensor_add(t_a[:], t_a[:], t_b[:])
                nc.vector.tensor_mul(t_b[:], rz, rz)
                nc.vector.tensor_add(d2[:], t_a[:], t_b[:])

                # theta_turns = d/10 (f32), split into bf16 coarse+residual
                thf = prol.tile([P, Q], f32)
                nc.scalar.activation(
                    thf[:], d2[:], SQRT, bias=zcol[:], scale=float(SC16 / FIX16**2)
                )
                thc = prol.tile([P, Q], bf16)
                nc.vector.tensor_copy(thc[:], thf[:])
                thr = prol.tile([P, Q], bf16)
                nc.vector.tensor_sub(thr[:], thf[:], thc[:])

                # interleaved [thc(8) | thr(8) | 1 | pad] per 8-q block, then
                # PE transposes into tmx.  Pad rows 17-63 of each window are
                # never read by the phase-matmul lhsT AP: left uninitialised.
                thiv = thi[:].rearrange("p (b j) -> p b j", j=64)
                nc.vector.tensor_copy(
                    thiv[:, :, 0:8], thc[:].rearrange("p (b j) -> p b j", j=8)
                )
                nc.vector.tensor_copy(
                    thiv[:, :, 8:16], thr[:].rearrange("p (b j) -> p b j", j=8)
                )
                nc.vector.memset(thiv[:, :, 16:17], 1.0)

                # warm the PE / overlap: first two MMW units only need h+W
                produce_mmw(0)
                produce_mmw(1)
                tr_src["thi"] = thi
                tr_src["idn"] = idn

                # q2 = c/d^4 = exp(-2*ln(d2) + ln(c)); reuse t_a/t_b
                lnq = t_a
                nc.scalar.activation(lnq[:], d2[:], LN, bias=zcol[:], scale=1.0)
                q2 = t_b
                nc.scalar.activation(q2[:], lnq[:], EXP, bias=lncol[:], scale=-2.0)

                rp_s = prol.tile([P, 3 * Q], f32)        # rp * (c/d^4)
                rps3 = rp_s[:].rearrange("p (x q) -> p x q", x=3)
                nc.vector.tensor_mul(
                    rps3, rp3, q2[:].unsqueeze(1).broadcast_to((P, 3, Q))
                )
                sq_s = prol.tile([P, 3 * Q], f32)        # rp^2 * (c/d^4)
                sqs3 = sq_s[:].rearrange("p (x q) -> p x q", x=3)
                nc.vector.tensor_mul(sqs3, rp3, rps3)
                xyq = d2                                 # x*y*(c/d^4), reuse d2
                nc.vector.tensor_mul(xyq[:], rx, rp_s[:, 1 * Q : 2 * Q])

                # m10: 10 plain (P,Q) muls, all on DVE (GPSIMD tensor ops
                # saturate SBUF bandwidth and throttle concurrent DVE 3-12x)
                for s in range(9):
                    al, be = divmod(s, 3)
                    nc.vector.tensor_mul(
                        m10[:, s * Q : (s + 1) * Q],
                        sq_s[:, al * Q : (al + 1) * Q],
                        rp_all[:, be * Q : (be + 1) * Q],
                    )
                nc.vector.tensor_mul(m10[:, 9 * Q : 10 * Q], xyq[:], rz)

            # ---- main loop: 2-deep software pipeline.  The DVE fold of
            # unit u-2 is issued FIRST so the PE's MMW(u) (which waits on
            # that fold freeing its PSUM buffer) never stalls on a
            # not-yet-enqueued DVE instruction; its MMAs go last. ----
            produce_phase(0)
            produce_phase(1)
            for u in range(2, NUNIT):
                wv = fold_unit(u - 2)
                produce_mmw(u)
                produce_phase(u)
                mma_unit(u - 2, wv)
            fold_mma_unit(NUNIT - 2)
            fold_mma_unit(NUNIT - 1)

            # ---- epilogue ----
            A_sb = const.tile([P, 20], f32)
            nc.vector.tensor_copy(A_sb[:], A_ps[:])
            nc.gpsimd.dma_start(out_ext, A_sb[:])

    nc.compile()
    return nc


def _get_nc():
    if "nc" not in _CACHE:
        _CACHE["nc"] = _build_nc()
    return _CACHE["nc"]


def kernel(h, rel_poss, W):
    from concourse.bass_utils import run_bass_kernel_spmd

    nc = _get_nc()

    h_pad = np.zeros((NTOT, K), dtype=np.float32)
    h_pad[:N_GLOBAL] = h
    rp_pad = np.ones((3, NTOT), dtype=np.float32)
    rp_pad[:, :N_GLOBAL] = rel_poss

    # Pre-transpose h to pair-stacked bf16 layout:
    # htp[i, 64*o + j, pi*128 + c] = h[i*NLOC + c*Q + 2*pi + o, j]
    Hc = h_pad.reshape(NCORES, P, Q, K).astype(ml_dtypes.bfloat16)
    ht = Hc.transpose(0, 3, 2, 1)                     # (i, j, q, c)
    htp = np.ascontiguousarray(
        ht.reshape(NCORES, K, Q // 2, 2, P).transpose(0, 3, 1, 2, 4)
    ).reshape(NCORES, P, (Q // 2) * P)

    wt = np.ascontiguousarray(W.T.astype(np.float32))   # wt[j,k] = W[k,j]
    blkw = np.zeros((P, P), dtype=np.float32)
    blkw[0:K, 0:K] = wt
    blkw[K:P, K:P] = wt
    blkw = blkw.astype(ml_dtypes.bfloat16)

    # KD: [17, 512] rhs for the PE phase matmul.
    # rows 0-7: (k+1)-diagonal for theta_c; rows 8-15: same for theta_r;
    # row 16: the 192.0 magic constant.
    kdm = np.zeros((P, 512), dtype=np.float32)
    kvals = np.arange(1, K + 1, dtype=np.float32)
    for base in (0, 64):
        for q in range(8):
            kdm[base + q, 64 * q : 64 * q + 64] = kvals
            kdm[base + 8 + q, 64 * q : 64 * q + 64] = kvals
        kdm[base + 16, :] = MAGIC
    kdm = kdm.astype(ml_dtypes.bfloat16)

    idn = np.eye(P, dtype=np.float32)

    in_maps = []
    for i in range(NCORES):
        in_maps.append(
            {
                "htp": htp[i],
                "rp": np.ascontiguousarray(rp_pad[:, i * NLOC : (i + 1) * NLOC]),
                "blkw": blkw,
                "kd": kdm,
                "idn": idn,
            }
        )

    res = run_bass_kernel_spmd(
        nc, in_maps, core_ids=list(range(NCORES)), trace=_CACHE.get("trace", False)
    )
    _CACHE["last_results"] = res
    acc = np.sum(
        [np.asarray(res.results[i]["out"], dtype=np.float32) for i in range(NCORES)],
        axis=0,
    )                                               # (128, 20)
    a20 = acc.reshape(P, 10, 2)
    A10 = a20[0:K, :, 0] + a20[K:P, :, 1]           # (64, 10)

    A = np.empty((K, 3, 3, 3), dtype=np.float32)
    for x in range(3):
        for y in range(3):
            for z in range(3):
                A[:, x, y, z] = A10[:, _MONO[tuple(sorted((x, y, z)))]]
    return A


if __name__ == "__main__":
    nc = _get_nc()
    print("build + compile OK")
